# revision 1
# baseline (speedup 1.0000x reference)
"""Trainium2 Bass kernel for two-stream cross-attention.

Reference computation (per batch b):
    qkv_s = x_s @ W_qkv_s ; split into q_s, k_s, v_s (16 heads x 64)
    dir1: out1 = softmax(q2 k1^T * scale) v1, merged @ W_out1 + b_out1
    dir2: out2 = softmax(q1 k2^T * scale) v2, merged @ W_out2 + b_out2

Sharding: 8 cores = 2 batches x 4 head-groups (4 heads each). Each core
computes q/k/v for its 4 heads (both streams), both attention directions,
and a partial output projection (row-block of W_out). Host sums the 4
partials per batch and adds the bias.

On-core pipeline (all matmuls bf16 with fp32 PSUM accumulation):
  A) x^T via DMA-transpose; qT/kT = W^T-as-lhsT matmuls; v via xT-as-lhsT.
     v is stored per 128-key-block with an appended ones column (v_ext).
  B) per (dir, head, q-tile): S^T tiles [128 keys, q] = kT^T-slice @ qT;
     exp on ScalarE (scale folded in; no max subtraction needed - scores
     are O(1) by construction) -> P^T bf16, which is exactly the lhsT
     layout the AV matmul needs. AV accumulates [q,64|rowsum] over key
     blocks; normalize by reciprocal rowsum; PE-transpose O -> O^T.
  C) out partial [n,1024] = O^T-as-lhsT @ W_out slice; DMA to DRAM f32.
"""

import os

import numpy as np
import ml_dtypes

import concourse.bass as bass
import concourse.mybir as mybir
import concourse.tile as tile
from concourse import bacc
from concourse.bass_utils import run_bass_kernel_spmd
from concourse.masks import make_identity

BF16 = mybir.dt.bfloat16
F32 = mybir.dt.float32

B, N, DIM = 2, 2048, 1024
HEADS, DH = 16, 64
HPC = 4                      # heads per core
HC = HPC * DH                # 256 inner columns per core
SCALE = DH ** -0.5
P = 128
FB = DIM // P                # 8 feature blocks
KB = N // P                  # 16 key blocks
QT = 1024                    # q-tile (attention inner loop)
NQT = N // QT                # 2 q-tiles

NCORES = 8

_NC = None
LAST_RESULTS = None


def _build():
    nc = bacc.Bacc(None, target_bir_lowering=False, debug=False, num_devices=NCORES)

    x1 = nc.dram_tensor("x1", [N, DIM], BF16, kind="ExternalInput")
    x2 = nc.dram_tensor("x2", [N, DIM], BF16, kind="ExternalInput")
    w1 = nc.dram_tensor("w1", [DIM, 3 * HC], BF16, kind="ExternalInput")
    w2 = nc.dram_tensor("w2", [DIM, 3 * HC], BF16, kind="ExternalInput")
    wo1 = nc.dram_tensor("wo1", [HC, DIM], BF16, kind="ExternalInput")
    wo2 = nc.dram_tensor("wo2", [HC, DIM], BF16, kind="ExternalInput")
    o1 = nc.dram_tensor("o1", [N, DIM], F32, kind="ExternalOutput")
    o2 = nc.dram_tensor("o2", [N, DIM], F32, kind="ExternalOutput")

    xs = [x1, x2]
    ws = [w1, w2]

    with tile.TileContext(nc) as tc:
        with (
            tc.tile_pool(name="const", bufs=1) as const_pool,
            tc.tile_pool(name="qkv", bufs=1) as qkv_pool,
            tc.tile_pool(name="wo", bufs=1) as wo_pool,
        ):
            identity = const_pool.tile([P, P], BF16)
            make_identity(nc, identity[:])

            # persistent per-stream q/k/v (bf16)
            qT = [qkv_pool.tile([P, 2, N], BF16, name=f"qT{s}") for s in range(2)]
            kT = [qkv_pool.tile([P, 2, N], BF16, name=f"kT{s}") for s in range(2)]
            vx = [qkv_pool.tile([P, KB, HPC, DH + 1], BF16, name=f"vx{s}")
                  for s in range(2)]
            wo_sb = [wo_pool.tile([P, 2, DIM], BF16, name=f"wo{s}") for s in range(2)]

            for s in range(2):
                nc.vector.memset(vx[s][:, :, :, DH], 1.0)
                for cb in range(2):
                    nc.sync.dma_start(
                        wo_sb[s][:, cb, :], [wo1, wo2][s][cb * P:(cb + 1) * P, :])

            # ---------------- Stage A: x^T, then qT/kT/v ----------------
            with (
                tc.tile_pool(name="xT", bufs=1) as xt_pool,
                tc.tile_pool(name="wqkv", bufs=1) as w_pool,
                tc.tile_pool(name="pqk", bufs=4, space="PSUM") as pqk_pool,
                tc.tile_pool(name="pv", bufs=2, space="PSUM") as pv_pool,
            ):
                xT = [xt_pool.tile([P, FB, N], BF16, name=f"xT{s}") for s in range(2)]
                w_sb = [w_pool.tile([P, FB, 3 * HC], BF16, name=f"w{s}")
                        for s in range(2)]
                for s in range(2):
                    for fb in range(FB):
                        nc.sync.dma_start(
                            xT[s][:, fb, :], xs[s][:, fb * P:(fb + 1) * P],
                            transpose=True)
                        nc.sync.dma_start(
                            w_sb[s][:, fb, :], ws[s][fb * P:(fb + 1) * P, :])

                for s in range(2):
                    # qT / kT : [c-block 128, n] = W[:, c-block]^T-as-lhsT @ xT
                    for which, off in ((qT[s], 0), (kT[s], HC)):
                        for cb in range(2):
                            for nt in range(4):
                                ps = pqk_pool.tile([P, 512], F32, name="pqk")
                                for fb in range(FB):
                                    nc.tensor.matmul(
                                        ps[:],
                                        w_sb[s][:, fb, off + cb * P:off + (cb + 1) * P],
                                        xT[s][:, fb, nt * 512:(nt + 1) * 512],
                                        start=(fb == 0), stop=(fb == FB - 1))
                                nc.vector.tensor_copy(
                                    which[:, cb, nt * 512:(nt + 1) * 512], ps[:])
                    # v : [n-block 128, 256] = xT-block-as-lhsT @ Wv
                    for kb in range(KB):
                        pv = pv_pool.tile([P, HC], F32, name="pv")
                        for fb in range(FB):
                            nc.tensor.matmul(
                                pv[:],
                                xT[s][:, fb, kb * P:(kb + 1) * P],
                                w_sb[s][:, fb, 2 * HC:3 * HC],
                                start=(fb == 0), stop=(fb == FB - 1))
                        nc.vector.tensor_copy(
                            vx[s][:, kb, :, 0:DH],
                            pv[:].rearrange("p (h d) -> p h d", h=HPC))

            # ---------------- Stage B: attention (both dirs) ----------------
            ot = [qkv_pool.tile([P, 2, N], BF16, name=f"ot{d}") for d in range(2)]

            with (
                tc.tile_pool(name="pt", bufs=2) as pt_pool,
                tc.tile_pool(name="osb", bufs=3) as osb_pool,
                tc.tile_pool(name="rec", bufs=3) as rec_pool,
                tc.tile_pool(name="pst", bufs=2, space="PSUM") as pst_pool,
                tc.tile_pool(name="pav", bufs=2, space="PSUM") as pav_pool,
                tc.tile_pool(name="ptr", bufs=2, space="PSUM") as ptr_pool,
            ):
                for d, (qs, ks) in enumerate(((1, 0), (0, 1))):
                    q_t, k_t, v_t = qT[qs], kT[ks], vx[ks]
                    for h in range(HPC):
                        po = (h % 2) * DH          # partition offset of head h
                        cb = h // 2                # c-block of head h
                        for qt in range(NQT):
                            pt = pt_pool.tile([P, KB, QT], BF16, name="pt")
                            for kb in range(KB):
                                st = pst_pool.tile([P, QT], F32, name="st")
                                for half in range(QT // 512):
                                    q0 = qt * QT + half * 512
                                    nc.tensor.matmul(
                                        st[:, half * 512:(half + 1) * 512],
                                        k_t[po:po + DH, cb, kb * P:(kb + 1) * P],
                                        q_t[po:po + DH, cb, q0:q0 + 512],
                                        start=True, stop=True)
                                nc.scalar.activation(
                                    pt[:, kb, :], st[:],
                                    mybir.ActivationFunctionType.Exp,
                                    scale=SCALE)
                            for m in range(QT // P):
                                oav = pav_pool.tile([P, DH + 1], F32, name="oav")
                                for kb in range(KB):
                                    nc.tensor.matmul(
                                        oav[:],
                                        pt[:, kb, m * P:(m + 1) * P],
                                        v_t[:, kb, h, :],
                                        start=(kb == 0), stop=(kb == KB - 1))
                                rec = rec_pool.tile([P, 1], F32, name="rec")
                                nc.vector.reciprocal(rec[:], oav[:, DH:DH + 1])
                                osb = osb_pool.tile([P, DH], BF16, name="osb")
                                nc.vector.tensor_scalar_mul(
                                    osb[:], oav[:, 0:DH], rec[:])
                                tr = ptr_pool.tile([DH, P], BF16, name="tr")
                                nc.tensor.transpose(tr[:], osb[:], identity[:])
                                col = (qt * (QT // P) + m) * P
                                nc.vector.tensor_copy(
                                    ot[d][po:po + DH, cb, col:col + P], tr[:])

            # ---------------- Stage C: output projection ----------------
            with (
                tc.tile_pool(name="out_sb", bufs=3) as out_pool,
                tc.tile_pool(name="pop", bufs=3, space="PSUM") as pop_pool,
            ):
                for d in range(2):
                    for m in range(N // P):
                        pop = pop_pool.tile([P, 512], F32, name="pop")
                        pop2 = pop_pool.tile([P, 512], F32, name="pop2")
                        for nb, ps in enumerate((pop, pop2)):
                            for cb in range(2):
                                nc.tensor.matmul(
                                    ps[:],
                                    ot[d][:, cb, m * P:(m + 1) * P],
                                    wo_sb[d][:, cb, nb * 512:(nb + 1) * 512],
                                    start=(cb == 0), stop=(cb == 1))
                        osb = out_pool.tile([P, DIM], F32, name="outsb")
                        nc.scalar.copy(osb[:, 0:512], pop[:])
                        nc.scalar.copy(osb[:, 512:DIM], pop2[:])
                        nc.sync.dma_start(
                            [o1, o2][d][m * P:(m + 1) * P, :], osb[:])

    nc.compile()
    return nc


def _shard_inputs(x1, x2, W_qkv1, W_qkv2, W_out1, W_out2):
    bf = ml_dtypes.bfloat16
    in_maps = []
    xs = [np.ascontiguousarray(x1).astype(bf), np.ascontiguousarray(x2).astype(bf)]
    w_full = [np.asarray(W_qkv1), np.asarray(W_qkv2)]
    wo_full = [np.asarray(W_out1), np.asarray(W_out2)]
    for cid in range(NCORES):
        b, g = divmod(cid, 4)
        cs = slice(g * HC, (g + 1) * HC)
        m = {}
        for s in range(2):
            m[f"x{s + 1}"] = xs[s][b]
            w = w_full[s]
            m[f"w{s + 1}"] = np.ascontiguousarray(np.concatenate(
                [w[:, 0:DIM][:, cs], w[:, DIM:2 * DIM][:, cs],
                 w[:, 2 * DIM:3 * DIM][:, cs]], axis=1)).astype(bf)
            m[f"wo{s + 1}"] = np.ascontiguousarray(wo_full[s][cs, :]).astype(bf)
        in_maps.append(m)
    return in_maps


def kernel(x1, x2, W_qkv1, W_qkv2, W_out1, b_out1, W_out2, b_out2):
    global _NC, LAST_RESULTS
    if _NC is None:
        _NC = _build()

    in_maps = _shard_inputs(x1, x2, W_qkv1, W_qkv2, W_out1, W_out2)
    trace = bool(os.environ.get("BASS_KERNEL_TRACE"))
    res = run_bass_kernel_spmd(_NC, in_maps, list(range(NCORES)), trace=trace)
    LAST_RESULTS = res

    out1 = np.zeros((B, N, DIM), np.float32)
    out2 = np.zeros((B, N, DIM), np.float32)
    for cid in range(NCORES):
        b = cid // 4
        out1[b] += res.results[cid]["o1"]
        out2[b] += res.results[cid]["o2"]
    out1 += np.asarray(b_out1, np.float32)
    out2 += np.asarray(b_out2, np.float32)
    return out1, out2



# revision 8
# speedup vs baseline: 1.1299x; 1.1299x over previous
"""Trainium2 Bass kernel for two-stream cross-attention.

Reference computation (per batch b):
    qkv_s = x_s @ W_qkv_s ; split into q_s, k_s, v_s (16 heads x 64)
    dir0: out1 = softmax(q2 k1^T * scale) v1, merged @ W_out1 + b_out1
    dir1: out2 = softmax(q1 k2^T * scale) v2, merged @ W_out2 + b_out2

Sharding: 8 cores = 2 batches x 4 head-groups (4 heads each). Each core
computes q/k/v for its 4 heads (both streams), both attention directions,
and a partial output projection (row-block of W_out). Host sums the 4
partials per batch and adds the bias.

Schedule notes (engines execute their queues in program order, so emission
order is the schedule):
  - x is transposed on the host; xT DMAs are plain contiguous transfers.
  - Stage A q/k use W as the stationary operand streamed against xT, so
    weight loads amortize over n=2048; outputs land as qT/kT [dh, n].
  - Attention keeps the S^T layout and consumes exp output per key block:
    QK(kb) -> exp([128,1024]) on ScalarE -> 8 AV steps accumulating into a
    single [128, 8, 128] psum tile holding all 8 m-accumulators (2 banks).
    start=True zeroes a whole 2KB psum zero-region, so only the first
    matmul per bank starts, and the normalize muls run in reverse m order
    so the next head's bank-zeroing start waits (via the m0 dependency on
    the in-order DVE) until every accumulator in the bank has been read.
  - exp on ScalarE (~1us per [128,1024] tile) is the floor engine, so all
    remaining stage-A work (q1/k2/v1/v2) and dir0's output projection are
    emitted as filler units inside the attention kb loops, keeping the PE
    busy during exp waits.
  - O tiles for a head pair share a [128q, 128] staging tile which is
    transposed SBUF->SBUF by the DMA xbar (no PE transpose, no PSUM).
  - Output projection accumulates in [128,512] psum tiles, DVE-copied to
    bf16 and DMAed as bf16 partials; the host reduces in f32.
"""

import os
from collections import deque

import numpy as np
import ml_dtypes

import concourse.bass as bass
import concourse.mybir as mybir
import concourse.tile as tile
from concourse import bacc
from concourse.bass_utils import run_bass_kernel_spmd

BF16 = mybir.dt.bfloat16
F32 = mybir.dt.float32

B, N, DIM = 2, 2048, 1024
HEADS, DH = 16, 64
HPC = 4                      # heads per core
HC = HPC * DH                # 256 inner columns per core
SCALE = DH ** -0.5
P = 128
FB = DIM // P                # 8 feature blocks
KB = N // P                  # 16 key blocks
QT = 1024                    # q-tile (attention inner loop)
NQT = N // QT                # 2 q-tiles
MT = QT // P                 # 8 m-tiles per q-tile

NCORES = 8

_NC = None
LAST_RESULTS = None


def _build():
    nc = bacc.Bacc(None, target_bir_lowering=False, debug=False, num_devices=NCORES)

    xt1 = nc.dram_tensor("xt1", [DIM, N], BF16, kind="ExternalInput")
    xt2 = nc.dram_tensor("xt2", [DIM, N], BF16, kind="ExternalInput")
    w1 = nc.dram_tensor("w1", [DIM, 3 * HC], BF16, kind="ExternalInput")
    w2 = nc.dram_tensor("w2", [DIM, 3 * HC], BF16, kind="ExternalInput")
    wo1 = nc.dram_tensor("wo1", [HC, DIM], BF16, kind="ExternalInput")
    wo2 = nc.dram_tensor("wo2", [HC, DIM], BF16, kind="ExternalInput")
    o1 = nc.dram_tensor("o1", [N, DIM], BF16, kind="ExternalOutput")
    o2 = nc.dram_tensor("o2", [N, DIM], BF16, kind="ExternalOutput")

    ws = [w1, w2]
    wos = [wo1, wo2]
    outs = [o1, o2]

    with tile.TileContext(nc) as tc:
        with (
            tc.tile_pool(name="qkv", bufs=1) as qkv_pool,
            tc.tile_pool(name="wo", bufs=1) as wo_pool,
            tc.tile_pool(name="xT", bufs=1) as xt_pool,
            tc.tile_pool(name="wqkv", bufs=1) as w_pool,
            tc.tile_pool(name="pt", bufs=4) as pt_pool,
            tc.tile_pool(name="rec", bufs=4) as rec_pool,
            tc.tile_pool(name="osb", bufs=10) as osb_pool,
            tc.tile_pool(name="outsb", bufs=3) as outsb_pool,
            tc.tile_pool(name="pA", bufs=2, space="PSUM") as pa_pool,
            tc.tile_pool(name="pst", bufs=2, space="PSUM") as pst_pool,
            tc.tile_pool(name="pav", bufs=1, space="PSUM") as pav_pool,
        ):
            # persistent per-stream q/k/v (bf16)
            qT = [qkv_pool.tile([P, 2, N], BF16, name=f"qT{s}") for s in range(2)]
            kT = [qkv_pool.tile([P, 2, N], BF16, name=f"kT{s}") for s in range(2)]
            vx = [qkv_pool.tile([P, KB, HPC, DH + 1], BF16, name=f"vx{s}")
                  for s in range(2)]
            ot = [qkv_pool.tile([P, 2, N], BF16, name=f"ot{d}") for d in range(2)]
            wo_sb = [wo_pool.tile([P, 2, DIM], BF16, name=f"wo{s}") for s in range(2)]
            xT = [xt_pool.tile([P, FB, N], BF16, name=f"xT{s}") for s in range(2)]
            w_sb = [w_pool.tile([P, FB, 3 * HC], BF16, name=f"w{s}")
                    for s in range(2)]

            # input DMAs: the two big streams ride separate queues
            for fb in range(FB):
                nc.sync.dma_start(w_sb[0][:, fb, :], w1[fb * P:(fb + 1) * P, :])
                nc.gpsimd.dma_start(w_sb[1][:, fb, :], w2[fb * P:(fb + 1) * P, :])
            for fb in range(FB):
                nc.sync.dma_start(xT[0][:, fb, :], xt1[fb * P:(fb + 1) * P, :])
                nc.gpsimd.dma_start(xT[1][:, fb, :], xt2[fb * P:(fb + 1) * P, :])
            for s in range(2):
                nc.vector.memset(vx[s][:, :, :, DH], 1.0)
                for cb in range(2):
                    nc.scalar.dma_start(
                        wo_sb[s][:, cb, :], wos[s][cb * P:(cb + 1) * P, :])

            # ---------- emit helpers ----------
            def emit_a_qk(s, woff, cb, dest, ch):
                # one unit: [dh-block 128, n-chunk 512] of qT/kT, W stationary
                ps = pa_pool.tile([P, 512], F32, name="psA", tag="u")
                for fb in range(FB):
                    nc.tensor.matmul(
                        ps[:],
                        w_sb[s][:, fb, woff + cb * P:woff + (cb + 1) * P],
                        xT[s][:, fb, ch * 512:(ch + 1) * 512],
                        start=(fb == 0), stop=(fb == FB - 1))
                nc.vector.tensor_copy(dest[:, cb, ch * 512:(ch + 1) * 512], ps[:])

            def emit_a_v(s, kb):
                pv = pa_pool.tile([P, 512], F32, name="psV", tag="u")
                for fb in range(FB):
                    nc.tensor.matmul(
                        pv[:, 0:HC],
                        xT[s][:, fb, kb * P:(kb + 1) * P],
                        w_sb[s][:, fb, 2 * HC:3 * HC],
                        start=(fb == 0), stop=(fb == FB - 1))
                nc.vector.tensor_copy(
                    vx[s][:, kb, :, 0:DH],
                    pv[:, 0:HC].rearrange("p (h d) -> p h d", h=HPC))

            def emit_av(oav, v_t, h, pt, kb):
                # 8 AV steps for one exp'd key block; only the first matmul
                # of each psum bank starts (zero-region semantics)
                for m in range(MT):
                    nc.tensor.matmul(
                        oav[:, m, 0:DH + 1],
                        pt[:, m * P:(m + 1) * P],
                        v_t[:, kb, h, :],
                        start=(kb == 0 and m % 4 == 0), stop=(kb == KB - 1),
                        skip_group_check=True)

            def emit_c(d, nb):
                # output projection for one 128-row block of n
                po = pa_pool.tile([P, 512], F32, name="psC", tag="u")
                po2 = pa_pool.tile([P, 512], F32, name="psC2", tag="u")
                osb = outsb_pool.tile([P, DIM], BF16, name="outsb")
                for chunk, ps in enumerate((po, po2)):
                    for cb in range(2):
                        nc.tensor.matmul(
                            ps[:],
                            ot[d][:, cb, nb * P:(nb + 1) * P],
                            wo_sb[d][:, cb, chunk * 512:(chunk + 1) * 512],
                            start=(cb == 0), stop=(cb == 1))
                    nc.vector.tensor_copy(
                        osb[:, chunk * 512:(chunk + 1) * 512], ps[:])
                nc.gpsimd.dma_start(outs[d][nb * P:(nb + 1) * P, :], osb[:])

            # filler queue: work units emitted between attention kb-steps so
            # the PE stays fed while ScalarE chews on exp
            filler = deque()

            def emit_filler(budget=1):
                for _ in range(budget):
                    if not filler:
                        return
                    fn, args = filler.popleft()
                    fn(*args)

            # upfront stage A: only what dir0 head-pair 0 needs (cb 0 of
            # k-stream0 / q-stream1); everything else is filler
            for ch in range(4):
                emit_a_qk(0, HC, 0, kT[0], ch)
            for ch in range(4):
                emit_a_qk(1, 0, 0, qT[1], ch)

            for kb in range(KB):
                filler.append((emit_a_v, (0, kb)))       # v1: dir0 AV needs it
            for ch in range(4):
                filler.append((emit_a_qk, (0, HC, 1, kT[0], ch)))
                filler.append((emit_a_qk, (1, 0, 1, qT[1], ch)))
            for cb in range(2):
                for ch in range(4):
                    filler.append((emit_a_qk, (0, 0, cb, qT[0], ch)))
                    filler.append((emit_a_qk, (1, HC, cb, kT[1], ch)))
            for kb in range(KB):
                filler.append((emit_a_v, (1, kb)))

            # ---------- attention ----------
            for d, (qs, ks) in enumerate(((1, 0), (0, 1))):
                q_t, k_t, v_t = qT[qs], kT[ks], vx[ks]
                for pair in range(2):          # head pairs (0,1), (2,3)
                    cb = pair
                    for qt in range(NQT):
                        osb_pair = [osb_pool.tile([P, P], BF16, name="osb")
                                    for _ in range(MT)]
                        for hh in range(2):
                            h = pair * 2 + hh
                            po = hh * DH
                            oav = pav_pool.tile([P, MT, P], F32, name="oav")
                            pending = None
                            for kb in range(KB):
                                st = pst_pool.tile([P, QT], F32, name="st")
                                for half in range(2):
                                    q0 = qt * QT + half * 512
                                    nc.tensor.matmul(
                                        st[:, half * 512:(half + 1) * 512],
                                        k_t[po:po + DH, cb, kb * P:(kb + 1) * P],
                                        q_t[po:po + DH, cb, q0:q0 + 512],
                                        start=True, stop=True)
                                pt = pt_pool.tile([P, QT], BF16, name="pt")
                                nc.scalar.activation(
                                    pt[:], st[:],
                                    mybir.ActivationFunctionType.Exp,
                                    scale=SCALE)
                                if pending is not None:
                                    emit_av(oav, v_t, h, *pending)
                                    emit_filler(1)
                                pending = (pt, kb)
                            emit_av(oav, v_t, h, *pending)
                            # normalize in reverse m order: the in-order DVE
                            # then guarantees every accumulator of a psum
                            # bank is read before the m0/m4 start-matmul of
                            # the next head re-zeroes the bank
                            recs = rec_pool.tile([P, MT], F32, name="recs")
                            nc.vector.reciprocal(recs[:], oav[:, :, DH:DH + 1])
                            for m in reversed(range(MT)):
                                nc.vector.tensor_scalar_mul(
                                    osb_pair[m][:, po:po + DH],
                                    oav[:, m, 0:DH], recs[:, m:m + 1])
                            emit_filler(1)
                        for m in range(MT):
                            col = qt * QT + m * P
                            nc.sync.dma_start(
                                ot[d][:, cb, col:col + P], osb_pair[m][:],
                                transpose=True)
                if d == 0:
                    for nb in range(N // P):
                        filler.append((emit_c, (0, nb)))
            # whatever filler remains, then dir1's projection
            emit_filler(len(filler))
            for nb in range(N // P):
                emit_c(1, nb)

    nc.compile()
    return nc


def _shard_inputs(x1, x2, W_qkv1, W_qkv2, W_out1, W_out2):
    bf = ml_dtypes.bfloat16
    in_maps = []
    # host-side transpose: [b, n, dim] -> per batch [dim, n], shared by the
    # 4 cores of that batch
    xts = [[np.ascontiguousarray(np.asarray(x[b]).astype(bf).T)
            for b in range(B)] for x in (x1, x2)]
    w_full = [np.asarray(W_qkv1), np.asarray(W_qkv2)]
    wo_full = [np.asarray(W_out1), np.asarray(W_out2)]
    w_cache = {}
    for cid in range(NCORES):
        b, g = divmod(cid, 4)
        cs = slice(g * HC, (g + 1) * HC)
        m = {}
        for s in range(2):
            m[f"xt{s + 1}"] = xts[s][b]
            if (s, g) not in w_cache:
                w = w_full[s]
                w_cache[(s, g)] = (
                    np.ascontiguousarray(np.concatenate(
                        [w[:, 0:DIM][:, cs], w[:, DIM:2 * DIM][:, cs],
                         w[:, 2 * DIM:3 * DIM][:, cs]], axis=1)).astype(bf),
                    np.ascontiguousarray(wo_full[s][cs, :]).astype(bf))
            m[f"w{s + 1}"], m[f"wo{s + 1}"] = w_cache[(s, g)]
        in_maps.append(m)
    return in_maps


def kernel(x1, x2, W_qkv1, W_qkv2, W_out1, b_out1, W_out2, b_out2):
    global _NC, LAST_RESULTS
    if _NC is None:
        _NC = _build()

    in_maps = _shard_inputs(x1, x2, W_qkv1, W_qkv2, W_out1, W_out2)
    trace = bool(os.environ.get("BASS_KERNEL_TRACE"))
    res = run_bass_kernel_spmd(_NC, in_maps, list(range(NCORES)), trace=trace)
    LAST_RESULTS = res

    out1 = np.zeros((B, N, DIM), np.float32)
    out2 = np.zeros((B, N, DIM), np.float32)
    for cid in range(NCORES):
        b = cid // 4
        out1[b] += np.asarray(res.results[cid]["o1"], np.float32)
        out2[b] += np.asarray(res.results[cid]["o2"], np.float32)
    out1 += np.asarray(b_out1, np.float32)
    out2 += np.asarray(b_out2, np.float32)
    return out1, out2
